# revision 23
# baseline (speedup 1.0000x reference)
"""GQA forward (B=2,N=2048,D=2048,H=32,KV=8,DH=64, causal) on 8 trn2 cores.

Sharding: 2-way data parallel over batch x 4-way tensor parallel over heads
(each core: 8 q-heads = 2 kv-heads, keeping group structure). Row-parallel
out-proj; the all-reduce over the 4 TP shards (+ bias) happens on host at
gather time.

v5 = v2 structure (HW-proven numerics paths only) + trace-driven fixes:
  - batched input DMAs: one dma_start per tensor phase (packets spread
    across all 16 DMA engines; x block 0 split across the sync+scalar
    HWDGE queues), single-tile weight/x buffers
  - dependency-free garbage warmup matmuls from t~0 sized to cover the
    input-DMA window
  - exp reads a strided 2-chunk view on partial key blocks (skips the
    never-written columns; slightly fewer ACT cycles, race-check clean)
  - outproj(0,1) deferred after attn(3) program-order: that window is
    ACT-bound (57us exp vs 37us attention matmul) and has no proj filler
  - final outproj psum evacuation alternates scalar/vector engines

Device kernel (per core):
  proj:  streaming projections from xT (host-pretransposed, bf16):
         Q^T (4 pair-slabs), K^T, V^T -> V (PE transpose) into vaug with
         ones columns (softmax row-sums for free in the ctx matmul)
  attn:  per pair (g, kv0/kv1), per 128-key block: two row-tiled scores
         matmuls S^T = K^T.T @ Q^T into one 2-bank PSUM tile, one exp on
         ACT (scale=1/sqrt(dh)), triangle-mask multiply on diagonal
         blocks, ctx^T accumulated per head in PSUM with row 64 = softmax
         denominator; normalize on the PSUM->SBUF copy.
  out:   out = ctx @ Wo_shard accumulated over 4 contraction chunks,
         written bf16; host sums the 4 TP partials + bias in fp32.
"""
import os
import sys

import numpy as np

if "/opt/trn_rl_repo" not in sys.path:
    sys.path.insert(0, "/opt/trn_rl_repo")

import ml_dtypes

import concourse.bacc as bacc
import concourse.tile as tile
from concourse import mybir
from concourse.bass_utils import run_bass_kernel_spmd
from concourse.masks import make_identity

F32 = mybir.dt.float32
BF16 = mybir.dt.bfloat16
EXP = mybir.ActivationFunctionType.Exp

B, N, D = 2, 2048, 2048
H, KV, DH = 32, 8, 64
G = H // KV                      # 4 q-heads per kv head
HPC, KVPC = 8, 2                 # heads / kv-heads per core
DQ = HPC * DH                    # 512 per-core q projection width
NBW = 512                        # q-block width for attention
NB = N // NBW                    # 4 q-blocks
DC = D // 128                    # 16 contraction chunks
NT = N // 128                    # 16 row tiles
NWARM = 34                       # garbage warmup matmuls (HAM + DMA cover)

_CACHED = {}


def _build():
    nc = bacc.Bacc("TRN2", target_bir_lowering=False, debug=False, num_devices=8)

    xT = nc.dram_tensor("xT", [D, N], BF16, kind="ExternalInput")
    Wq = nc.dram_tensor("Wq", [D, DQ], BF16, kind="ExternalInput")
    Wk = nc.dram_tensor("Wk", [D, KVPC * DH], BF16, kind="ExternalInput")
    Wv = nc.dram_tensor("Wv", [D, KVPC * DH], BF16, kind="ExternalInput")
    Wo = nc.dram_tensor("Wo", [DQ, D], BF16, kind="ExternalInput")
    OUT = nc.dram_tensor("out", [N, D], BF16, kind="ExternalOutput")

    scale = 1.0 / np.sqrt(DH)

    with tile.TileContext(nc) as tc:
        with (
            tc.tile_pool(name="persist", bufs=1) as pp,
            tc.tile_pool(name="vt", bufs=2) as vtp,
            tc.tile_pool(name="pt", bufs=6) as ptp,
            tc.tile_pool(name="outs", bufs=2) as outp,
            tc.tile_pool(name="small", bufs=3) as smp,
            tc.tile_pool(name="ps", bufs=1, space="PSUM") as psp,
        ):
            # ---- PE warmup: dependency-free garbage matmuls keep the HAM
            # clock warming while the batched input DMAs stream in ----
            junk = pp.tile([128, NBW], BF16, tag="junk")
            nc.vector.memset(junk[:], 0.0)
            wu_ps = psp.tile([128, NBW], F32, tag="sps", name="wu", bufs=2)
            for _ in range(NWARM):
                nc.tensor.matmul(wu_ps[:], junk[:, 0:128], junk[:],
                                 start=True, stop=True)

            # ---- batched input DMAs: one instruction per tensor phase ----
            xs_all = pp.tile([128, DC * N], BF16, tag="xsall")
            wk_all = pp.tile([128, DC * KVPC * DH], BF16, tag="wkall")
            wv_all = pp.tile([128, DC * KVPC * DH], BF16, tag="wvall")
            wq_all = pp.tile([128, DC * DQ], BF16, tag="wqall")
            wo_all = pp.tile([128, 4 * D], BF16, tag="woall")

            xs_v = xs_all[:].rearrange("p (dc n) -> p dc n", dc=DC)
            xT_v = xT[:, :].rearrange("(dc p) n -> p dc n", p=128)
            # critical path first: wk + x block 0 + wq gate proj(0).
            # Asymmetric x0 split: scalar's small quarter clears fast so
            # wq (behind it in the ring) lands right as proj_k finishes;
            # x block 1 is its own transfer so proj(1) isn't gated on the
            # 4MB bulk. ~375GB/s aggregate across the 16 DMA engines.
            nc.sync.dma_start(
                out=wk_all[:].rearrange("p (dc m) -> p dc m", dc=DC),
                in_=Wk[:, :].rearrange("(dc p) m -> p dc m", p=128))
            nc.sync.dma_start(out=xs_v[:, 0:11, 0:NBW],
                              in_=xT_v[:, 0:11, 0:NBW])
            nc.scalar.dma_start(out=xs_v[:, 11:DC, 0:NBW],
                                in_=xT_v[:, 11:DC, 0:NBW])
            nc.scalar.dma_start(
                out=wq_all[:].rearrange("p (dc m) -> p dc m", dc=DC),
                in_=Wq[:, :].rearrange("(dc p) m -> p dc m", p=128))
            nc.sync.dma_start(out=xs_v[:, :, NBW:2 * NBW],
                              in_=xT_v[:, :, NBW:2 * NBW])
            nc.sync.dma_start(
                out=wv_all[:].rearrange("p (dc m) -> p dc m", dc=DC),
                in_=Wv[:, :].rearrange("(dc p) m -> p dc m", p=128))
            nc.sync.dma_start(out=xs_v[:, :, 2 * NBW:N],
                              in_=xT_v[:, :, 2 * NBW:N])
            nc.scalar.dma_start(
                out=wo_all[:].rearrange("p (j d) -> p j d", j=4),
                in_=Wo[:, :].rearrange("(j p) d -> p j d", p=128))

            def xs(dc, nb):
                return xs_all[:, dc * N + nb * NBW:dc * N + (nb + 1) * NBW]

            def wk_sb(dc):
                return wk_all[:, dc * 128:(dc + 1) * 128]

            def wv_sb(dc):
                return wv_all[:, dc * 128:(dc + 1) * 128]

            def wq_sb(dc, s):
                return wq_all[:, dc * DQ + s * 128:dc * DQ + (s + 1) * 128]

            def wo_sb(j, ob):
                return wo_all[:, j * D + ob * NBW:j * D + (ob + 1) * NBW]

            # ---- persistent sbuf state ----
            ident_f = pp.tile([128, 128], F32, tag="identf")
            make_identity(nc, ident_f[:])
            ident = pp.tile([128, 128], BF16, tag="ident")
            nc.vector.tensor_copy(ident[:], ident_f[:])

            # triangle mask: tri[r, j] = 1 if j >= r else 0
            tri_f = pp.tile([128, 128], F32, tag="trif")
            nc.gpsimd.memset(tri_f[:], 1.0)
            nc.gpsimd.affine_select(
                out=tri_f[:], in_=tri_f[:],
                compare_op=mybir.AluOpType.is_ge,
                fill=0.0, base=0,
                pattern=[[1, 128]],
                channel_multiplier=-1,
            )
            tri = pp.tile([128, 128], BF16, tag="tri")
            nc.vector.tensor_copy(tri[:], tri_f[:])

            ones_t = pp.tile([128, DH], BF16, tag="ones")
            nc.vector.memset(ones_t[:], 1.0)

            # q slabs: slab s = [kv0 g=s (rows 0:64) | kv1 g=s (rows 64:128)]
            qt = [pp.tile([128, N], BF16, tag=f"qt{s}", name=f"qt{s}")
                  for s in range(4)]
            kt = pp.tile([128, N], BF16, tag="kt")
            # vaug[m]: [0:64]=v_kv0, 64=ones, [65:129]=v_kv1, 129=ones
            vaug = [pp.tile([128, 2 * (DH + 1)], BF16, tag=f"va{m}",
                            name=f"va{m}") for m in range(NT)]
            for m in range(NT):
                nc.vector.memset(vaug[m][:], 1.0)
            ctxT = [pp.tile([128, N], BF16, tag=f"ct{j}", name=f"ct{j}")
                    for j in range(4)]

            # ---- projection helpers (chains alternate scr/prps so chain
            # i+1 never waits on chain i's evacuation cast) ----
            def proj_k(nb):
                ncol = slice(nb * NBW, (nb + 1) * NBW)
                k_ps = psp.tile([128, NBW], F32, tag="scr", name="kps")
                for dc in range(DC):
                    nc.tensor.matmul(k_ps[:], wk_sb(dc), xs(dc, nb),
                                     start=(dc == 0), stop=(dc == DC - 1))
                nc.vector.tensor_copy(kt[:, ncol], k_ps[:])

            def proj_v(nb):
                v_ps = psp.tile([128, NBW], F32, tag="prps", name="vps")
                for dc in range(DC):
                    nc.tensor.matmul(v_ps[:], wv_sb(dc), xs(dc, nb),
                                     start=(dc == 0), stop=(dc == DC - 1))
                vts = vtp.tile([128, NBW], BF16, tag="vts")
                nc.vector.tensor_copy(vts[:], v_ps[:])
                for i in range(NBW // 128):
                    mt = nb * (NBW // 128) + i
                    tp = psp.tile([128, 128], BF16, tag="scr", name="tps")
                    nc.tensor.transpose(tp[:], vts[:, i * 128:(i + 1) * 128],
                                        ident[:])
                    nc.vector.tensor_copy(vaug[mt][:, 0:DH], tp[:, 0:DH])
                    nc.vector.tensor_copy(vaug[mt][:, DH + 1:2 * DH + 1],
                                          tp[:, DH:2 * DH])

            def proj_q(nb, s):
                tg = "prps" if s % 2 == 0 else "scr"
                ncol = slice(nb * NBW, (nb + 1) * NBW)
                q_ps = psp.tile([128, NBW], F32, tag=tg, name="qps")
                for dc in range(DC):
                    nc.tensor.matmul(q_ps[:], wq_sb(dc, s), xs(dc, nb),
                                     start=(dc == 0), stop=(dc == DC - 1))
                nc.vector.tensor_copy(qt[s][:, ncol], q_ps[:])

            # ---- attention ----
            def emit_norm(c_ps, j, par, q0):
                # ctx^T rows /= row 64 (ones-col sums). Broadcast the sums
                # from psum partition 64 to 0:64 with a K=1 ones matmul.
                # The broadcast lands in the sps rotation (not scr) so the
                # proj/outproj chains keep two conflict-free banks.
                lrow = smp.tile([128, NBW], BF16, tag="lrow", name="lrow")
                nc.vector.tensor_copy(lrow[DH:DH + 1, :], c_ps[DH:DH + 1, :])
                rb_t = psp.tile([128, 2 * NBW], F32, tag="sps", name="rbps",
                                bufs=2)
                rb_ps = rb_t[0:DH, 0:NBW]
                nc.tensor.matmul(rb_ps, ones_t[DH:DH + 1, 0:DH],
                                 lrow[DH:DH + 1, :], start=True, stop=True)
                rb = smp.tile([DH, NBW], F32, tag="rb", name="rb")
                nc.vector.reciprocal_approx_fast(out=rb[:], in_=rb_ps)
                if par == 0:
                    nc.vector.tensor_mul(ctxT[j][0:DH, q0:q0 + NBW],
                                         c_ps[0:DH, :], rb[:])
                else:
                    tmp = smp.tile([DH, NBW], BF16, tag="ctmp", name="ctmp")
                    nc.vector.tensor_mul(tmp[:], c_ps[0:DH, :], rb[:])
                    nc.sync.dma_start(out=ctxT[j][DH:2 * DH, q0:q0 + NBW],
                                      in_=tmp[:])

            def attn(nb):
                q0 = nb * NBW
                n_mb = 4 * nb + 4
                for s in range(4):
                    j, par = s // 2, s % 2
                    c0 = psp.tile([DH + 1, NBW], F32, tag="cps", name="c0",
                                  bufs=2)
                    c1 = psp.tile([DH + 1, NBW], F32, tag="cps", name="c1",
                                  bufs=2)
                    # exp groups: each group = one sps tile + one activation.
                    # The two smallest diagonal blocks (w=256,128) share a
                    # tile: [mb2kv0 0:256|mb3kv0 256:384|mb2kv1 512:768|
                    # mb3kv1 768:896] — each matmul output stays in one bank.
                    groups = [[mb] for mb in range(n_mb - 2)]
                    groups.append([n_mb - 2, n_mb - 1])
                    for grp in groups:
                        place = []
                        if len(grp) == 1:
                            mb = grp[0]
                            off = max(0, mb * 128 - q0)
                            w = NBW - off
                            place.append((mb, off, w, 0, NBW))
                            vw = w
                        else:
                            # kv0 regions stay in bank 0, kv1 in bank 1, so
                            # the concurrently-draining row-tiled pair never
                            # writes the same psum bank.
                            mb2, mb3 = grp
                            place.append((mb2, 256, 256, 0, NBW))
                            place.append((mb3, 384, 128, 256, NBW + 256))
                            vw = 384
                        sp = psp.tile([128, 2 * NBW], F32, tag="sps",
                                      name="sps", bufs=2)
                        for mb, off, w, ca, cb in place:
                            m0 = mb * 128
                            nc.tensor.matmul(
                                sp[:, ca:ca + w],
                                kt[0:DH, m0:m0 + 128],
                                qt[s][0:DH, q0 + off:q0 + NBW],
                                start=True, stop=True)
                            nc.tensor.matmul(
                                sp[:, cb:cb + w],
                                kt[DH:128, m0:m0 + 128],
                                qt[s][DH:128, q0 + off:q0 + NBW],
                                start=True, stop=True)
                        p = ptp.tile([128, 2 * NBW], BF16, tag="pt",
                                     name="pt")
                        if vw == NBW:
                            nc.scalar.activation(p[:, 0:2 * NBW],
                                                 sp[:, 0:2 * NBW],
                                                 EXP, scale=float(scale))
                        else:
                            # strided 2-chunk view skips the stale columns
                            # [vw:NBW] (never written in this tile)
                            sp_v = sp[:].rearrange(
                                "p (c k) -> p c k", c=2)[:, :, 0:vw]
                            p_v = p[:].rearrange(
                                "p (c k) -> p c k", c=2)[:, :, 0:vw]
                            nc.scalar.activation(p_v, sp_v, EXP,
                                                 scale=float(scale))
                        for mb, off, w, ca, cb in place:
                            if mb >= 4 * nb:  # diagonal block
                                nc.vector.tensor_mul(p[:, ca:ca + 128],
                                                     p[:, ca:ca + 128], tri[:])
                                nc.vector.tensor_mul(p[:, cb:cb + 128],
                                                     p[:, cb:cb + 128], tri[:])
                            st, sp_ = (mb == 0), (mb == n_mb - 1)
                            nc.tensor.matmul(c0[:, off:NBW],
                                             vaug[mb][:, 0:DH + 1],
                                             p[:, ca:ca + w],
                                             start=st, stop=sp_)
                            nc.tensor.matmul(c1[:, off:NBW],
                                             vaug[mb][:, DH + 1:2 * (DH + 1)],
                                             p[:, cb:cb + w],
                                             start=st, stop=sp_)
                    emit_norm(c0, j, par, q0)
                    emit_norm(c1, 2 + j, par, q0)

            # ---- out projection ----
            def outproj(nb):
                if nb == NB - 1:
                    # pure tail (attention done): all four freed psum banks
                    # accumulate in parallel, j-outer so each ctxT slice is
                    # loaded once per 4 matmuls; per-chunk DMAs drain early;
                    # copies alternate scalar/vector to halve the drain.
                    for nt in range(4 * nb, 4 * nb + 4):
                        o_sb = outp.tile([128, D], BF16, tag="osb",
                                         name="osb")
                        ops = []
                        for tg in ("cps", "cps", "scr", "prps"):
                            ops.append(psp.tile(
                                [128, NBW], F32, tag=tg, name="ops",
                                bufs=2 if tg == "cps" else 1))
                        for j in range(4):
                            for ob in range(4):
                                nc.tensor.matmul(
                                    ops[ob][:],
                                    ctxT[j][:, nt * 128:(nt + 1) * 128],
                                    wo_sb(j, ob),
                                    start=(j == 0), stop=(j == 3))
                        for ob in range(4):
                            dst = o_sb[:, ob * NBW:(ob + 1) * NBW]
                            if ob % 2 == 0:
                                nc.scalar.copy(dst, ops[ob][:])
                            else:
                                nc.vector.tensor_copy(dst, ops[ob][:])
                            nc.sync.dma_start(
                                out=OUT[nt * 128:(nt + 1) * 128,
                                        ob * NBW:(ob + 1) * NBW],
                                in_=o_sb[:, ob * NBW:(ob + 1) * NBW])
                    return
                # filler for the attn(3) window (no proj runs there);
                # chains alternate scr/prps banks.
                u = 0
                for nt in range(4 * nb, 4 * nb + 4):
                    o_sb = outp.tile([128, D], BF16, tag="osb", name="osb")
                    for ob in range(4):
                        tg = "scr" if u % 2 == 0 else "prps"
                        u += 1
                        o_ps = psp.tile([128, NBW], F32, tag=tg, name="ops",
                                        bufs=1)
                        for j in range(4):
                            nc.tensor.matmul(
                                o_ps[:],
                                ctxT[j][:, nt * 128:(nt + 1) * 128],
                                wo_sb(j, ob),
                                start=(j == 0), stop=(j == 3))
                        nc.vector.tensor_copy(
                            o_sb[:, ob * NBW:(ob + 1) * NBW], o_ps[:])
                    nc.sync.dma_start(out=OUT[nt * 128:(nt + 1) * 128, :],
                                      in_=o_sb[:])

            # ---- program: per-q-block pipeline ----
            # proj(nb+1) sits after attn(nb) in program order (= lower
            # scheduler priority) so it fills PE idle while ACT churns.
            # outproj(0..2) deferred after attn(3): that window is ACT-bound
            # and otherwise short of PE filler.
            proj_k(0)
            for s in range(4):
                proj_q(0, s)
            proj_v(0)
            for nb in range(NB):
                attn(nb)
                if nb + 1 < NB:
                    proj_k(nb + 1)
                    for s in range(4):
                        proj_q(nb + 1, s)
                    proj_v(nb + 1)
            outproj(0)
            outproj(1)
            outproj(2)
            outproj(NB - 1)

    nc.compile()
    return nc


def kernel(x, Wq, Wk, Wv, Wo, bo):
    x = np.asarray(x, dtype=np.float32)
    Wq = np.asarray(Wq, dtype=np.float32)
    Wk = np.asarray(Wk, dtype=np.float32)
    Wv = np.asarray(Wv, dtype=np.float32)
    Wo = np.asarray(Wo, dtype=np.float32)
    bo = np.asarray(bo, dtype=np.float32)
    bf = ml_dtypes.bfloat16

    if "nc" not in _CACHED:
        _CACHED["nc"] = _build()
    nc = _CACHED["nc"]

    in_maps = []
    for c in range(8):
        b, t = c // 4, c % 4
        xT = np.ascontiguousarray(x[b].T).astype(bf)
        # q slab s holds [kv-head 2t head g=s | kv-head 2t+1 head g=s]
        qcols = []
        for s in range(4):
            for kvl in range(KVPC):
                h = (2 * t + kvl) * G + s
                qcols.append(Wq[:, h * DH:(h + 1) * DH])
        wq_c = np.ascontiguousarray(np.concatenate(qcols, axis=1)).astype(bf)
        wk_c = np.ascontiguousarray(Wk[:, t * 128:(t + 1) * 128]).astype(bf)
        wv_c = np.ascontiguousarray(Wv[:, t * 128:(t + 1) * 128]).astype(bf)
        wo_c = np.ascontiguousarray(Wo[t * DQ:(t + 1) * DQ, :]).astype(bf)
        in_maps.append({"xT": xT, "Wq": wq_c, "Wk": wk_c, "Wv": wv_c,
                        "Wo": wo_c})

    trace = bool(int(os.environ.get("GQA_TRACE", "0")))
    kwargs = {}
    if trace:
        import tempfile
        td = os.environ.get("GQA_TRACE_DIR") or tempfile.mkdtemp(prefix="gqa_")
        kwargs = dict(trace=True, tmpdir=td)
    res = run_bass_kernel_spmd(nc, in_maps, list(range(8)), **kwargs)
    _CACHED["last_result"] = res

    out = np.empty((B, N, D), dtype=np.float32)
    for b in range(B):
        acc = res.results[4 * b]["out"].astype(np.float32)
        for t in range(1, 4):
            acc = acc + res.results[4 * b + t]["out"].astype(np.float32)
        out[b] = acc + bo[None, :]
    return out


# revision 26
# speedup vs baseline: 1.0302x; 1.0302x over previous
"""GQA forward (B=2,N=2048,D=2048,H=32,KV=8,DH=64, causal) on 8 trn2 cores.

Sharding: 2-way data parallel over batch x 4-way tensor parallel over heads
(each core: 8 q-heads = 2 kv-heads, keeping group structure). Row-parallel
out-proj; the all-reduce over the 4 TP shards (+ bias) happens on host at
gather time.

v5 = v2 structure (HW-proven numerics paths only) + trace-driven fixes:
  - batched input DMAs: one dma_start per tensor phase (packets spread
    across all 16 DMA engines; x block 0 split across the sync+scalar
    HWDGE queues), single-tile weight/x buffers
  - dependency-free garbage warmup matmuls from t~0 sized to cover the
    input-DMA window
  - exp reads a strided 2-chunk view on partial key blocks (skips the
    never-written columns; slightly fewer ACT cycles, race-check clean)
  - outproj(0,1) deferred after attn(3) program-order: that window is
    ACT-bound (57us exp vs 37us attention matmul) and has no proj filler
  - final outproj psum evacuation alternates scalar/vector engines

Device kernel (per core):
  proj:  streaming projections from xT (host-pretransposed, bf16):
         Q^T (4 pair-slabs), K^T, V^T -> V (PE transpose) into vaug with
         ones columns (softmax row-sums for free in the ctx matmul)
  attn:  per pair (g, kv0/kv1), per 128-key block: two row-tiled scores
         matmuls S^T = K^T.T @ Q^T into one 2-bank PSUM tile, one exp on
         ACT (scale=1/sqrt(dh)), triangle-mask multiply on diagonal
         blocks, ctx^T accumulated per head in PSUM with row 64 = softmax
         denominator; normalize on the PSUM->SBUF copy.
  out:   out = ctx @ Wo_shard accumulated over 4 contraction chunks,
         written bf16; host sums the 4 TP partials + bias in fp32.
"""
import os
import sys

import numpy as np

if "/opt/trn_rl_repo" not in sys.path:
    sys.path.insert(0, "/opt/trn_rl_repo")

import ml_dtypes

import concourse.bacc as bacc
import concourse.tile as tile
from concourse import mybir
from concourse.bass_utils import run_bass_kernel_spmd
from concourse.masks import make_identity

F32 = mybir.dt.float32
BF16 = mybir.dt.bfloat16
EXP = mybir.ActivationFunctionType.Exp

B, N, D = 2, 2048, 2048
H, KV, DH = 32, 8, 64
G = H // KV                      # 4 q-heads per kv head
HPC, KVPC = 8, 2                 # heads / kv-heads per core
DQ = HPC * DH                    # 512 per-core q projection width
NBW = 512                        # q-block width for attention
NB = N // NBW                    # 4 q-blocks
DC = D // 128                    # 16 contraction chunks
NT = N // 128                    # 16 row tiles
NWARM = 24                       # garbage warmup matmuls (HAM + DMA cover)

_CACHED = {}


def _build():
    nc = bacc.Bacc("TRN2", target_bir_lowering=False, debug=False, num_devices=8)

    xT = nc.dram_tensor("xT", [D, N], BF16, kind="ExternalInput")
    Wq = nc.dram_tensor("Wq", [D, DQ], BF16, kind="ExternalInput")
    Wk = nc.dram_tensor("Wk", [D, KVPC * DH], BF16, kind="ExternalInput")
    Wv = nc.dram_tensor("Wv", [D, KVPC * DH], BF16, kind="ExternalInput")
    Wo = nc.dram_tensor("Wo", [DQ, D], BF16, kind="ExternalInput")
    OUT = nc.dram_tensor("out", [N, D], BF16, kind="ExternalOutput")

    scale = 1.0 / np.sqrt(DH)

    with tile.TileContext(nc) as tc:
        with (
            tc.tile_pool(name="persist", bufs=1) as pp,
            tc.tile_pool(name="vt", bufs=2) as vtp,
            tc.tile_pool(name="pt", bufs=6) as ptp,
            tc.tile_pool(name="outs", bufs=2) as outp,
            tc.tile_pool(name="small", bufs=3) as smp,
            tc.tile_pool(name="ps", bufs=1, space="PSUM") as psp,
        ):
            # ---- PE warmup: dependency-free garbage matmuls keep the HAM
            # clock warming while the batched input DMAs stream in ----
            junk = pp.tile([128, NBW], BF16, tag="junk")
            nc.vector.memset(junk[:], 0.0)
            wu_ps = psp.tile([128, NBW], F32, tag="sps", name="wu", bufs=2)
            for _ in range(NWARM):
                nc.tensor.matmul(wu_ps[:], junk[:, 0:128], junk[:],
                                 start=True, stop=True)

            # ---- batched input DMAs: one instruction per tensor phase ----
            xs_all = pp.tile([128, DC * N], BF16, tag="xsall")
            wk_all = pp.tile([128, DC * KVPC * DH], BF16, tag="wkall")
            wv_all = pp.tile([128, DC * KVPC * DH], BF16, tag="wvall")
            wq_all = pp.tile([128, DC * DQ], BF16, tag="wqall")
            wo_all = pp.tile([128, 4 * D], BF16, tag="woall")

            xs_v = xs_all[:].rearrange("p (dc n) -> p dc n", dc=DC)
            xT_v = xT[:, :].rearrange("(dc p) n -> p dc n", p=128)
            # critical path first: wk + x block 0 + wq gate proj(0).
            # Asymmetric x0 split: scalar's small quarter clears fast so
            # wq (behind it in the ring) lands right as proj_k finishes;
            # x block 1 is its own transfer so proj(1) isn't gated on the
            # 4MB bulk. ~375GB/s aggregate across the 16 DMA engines.
            nc.sync.dma_start(
                out=wk_all[:].rearrange("p (dc m) -> p dc m", dc=DC),
                in_=Wk[:, :].rearrange("(dc p) m -> p dc m", p=128))
            # x0 chunked so proj_k's accumulation chain is paced by DMA
            # arrival (dc 0-3 lands ~14us) instead of one big gate
            for d0, d1 in ((0, 4), (4, 8), (8, 11)):
                nc.sync.dma_start(out=xs_v[:, d0:d1, 0:NBW],
                                  in_=xT_v[:, d0:d1, 0:NBW])
            nc.scalar.dma_start(out=xs_v[:, 11:DC, 0:NBW],
                                in_=xT_v[:, 11:DC, 0:NBW])
            nc.scalar.dma_start(
                out=wq_all[:].rearrange("p (dc m) -> p dc m", dc=DC),
                in_=Wq[:, :].rearrange("(dc p) m -> p dc m", p=128))
            nc.sync.dma_start(out=xs_v[:, :, NBW:2 * NBW],
                              in_=xT_v[:, :, NBW:2 * NBW])
            nc.sync.dma_start(
                out=wv_all[:].rearrange("p (dc m) -> p dc m", dc=DC),
                in_=Wv[:, :].rearrange("(dc p) m -> p dc m", p=128))
            nc.sync.dma_start(out=xs_v[:, :, 2 * NBW:N],
                              in_=xT_v[:, :, 2 * NBW:N])
            nc.scalar.dma_start(
                out=wo_all[:].rearrange("p (j d) -> p j d", j=4),
                in_=Wo[:, :].rearrange("(j p) d -> p j d", p=128))

            def xs(dc, nb):
                return xs_all[:, dc * N + nb * NBW:dc * N + (nb + 1) * NBW]

            def wk_sb(dc):
                return wk_all[:, dc * 128:(dc + 1) * 128]

            def wv_sb(dc):
                return wv_all[:, dc * 128:(dc + 1) * 128]

            def wq_sb(dc, s):
                return wq_all[:, dc * DQ + s * 128:dc * DQ + (s + 1) * 128]

            def wo_sb(j, ob):
                return wo_all[:, j * D + ob * NBW:j * D + (ob + 1) * NBW]

            # ---- persistent sbuf state ----
            ident_f = pp.tile([128, 128], F32, tag="identf")
            make_identity(nc, ident_f[:])
            ident = pp.tile([128, 128], BF16, tag="ident")
            nc.vector.tensor_copy(ident[:], ident_f[:])

            # triangle mask: tri[r, j] = 1 if j >= r else 0
            tri_f = pp.tile([128, 128], F32, tag="trif")
            nc.gpsimd.memset(tri_f[:], 1.0)
            nc.gpsimd.affine_select(
                out=tri_f[:], in_=tri_f[:],
                compare_op=mybir.AluOpType.is_ge,
                fill=0.0, base=0,
                pattern=[[1, 128]],
                channel_multiplier=-1,
            )
            tri = pp.tile([128, 128], BF16, tag="tri")
            nc.vector.tensor_copy(tri[:], tri_f[:])

            ones_t = pp.tile([128, DH], BF16, tag="ones")
            nc.vector.memset(ones_t[:], 1.0)

            # q slabs: slab s = [kv0 g=s (rows 0:64) | kv1 g=s (rows 64:128)]
            qt = [pp.tile([128, N], BF16, tag=f"qt{s}", name=f"qt{s}")
                  for s in range(4)]
            kt = pp.tile([128, N], BF16, tag="kt")
            # vaug[m]: [0:64]=v_kv0, 64=ones, [65:129]=v_kv1, 129=ones
            vaug = [pp.tile([128, 2 * (DH + 1)], BF16, tag=f"va{m}",
                            name=f"va{m}") for m in range(NT)]
            for m in range(NT):
                nc.vector.memset(vaug[m][:], 1.0)
            ctxT = [pp.tile([128, N], BF16, tag=f"ct{j}", name=f"ct{j}")
                    for j in range(4)]

            # ---- projection helpers (chains alternate scr/prps so chain
            # i+1 never waits on chain i's evacuation cast) ----
            def proj_k(nb):
                ncol = slice(nb * NBW, (nb + 1) * NBW)
                k_ps = psp.tile([128, NBW], F32, tag="scr", name="kps")
                for dc in range(DC):
                    nc.tensor.matmul(k_ps[:], wk_sb(dc), xs(dc, nb),
                                     start=(dc == 0), stop=(dc == DC - 1))
                nc.vector.tensor_copy(kt[:, ncol], k_ps[:])

            def proj_v(nb):
                v_ps = psp.tile([128, NBW], F32, tag="prps", name="vps")
                for dc in range(DC):
                    nc.tensor.matmul(v_ps[:], wv_sb(dc), xs(dc, nb),
                                     start=(dc == 0), stop=(dc == DC - 1))
                vts = vtp.tile([128, NBW], BF16, tag="vts")
                nc.vector.tensor_copy(vts[:], v_ps[:])
                for i in range(NBW // 128):
                    mt = nb * (NBW // 128) + i
                    tp = psp.tile([128, 128], BF16, tag="scr", name="tps")
                    nc.tensor.transpose(tp[:], vts[:, i * 128:(i + 1) * 128],
                                        ident[:])
                    nc.vector.tensor_copy(vaug[mt][:, 0:DH], tp[:, 0:DH])
                    nc.vector.tensor_copy(vaug[mt][:, DH + 1:2 * DH + 1],
                                          tp[:, DH:2 * DH])

            def proj_q(nb, s):
                tg = "prps" if s % 2 == 0 else "scr"
                ncol = slice(nb * NBW, (nb + 1) * NBW)
                q_ps = psp.tile([128, NBW], F32, tag=tg, name="qps")
                for dc in range(DC):
                    nc.tensor.matmul(q_ps[:], wq_sb(dc, s), xs(dc, nb),
                                     start=(dc == 0), stop=(dc == DC - 1))
                nc.vector.tensor_copy(qt[s][:, ncol], q_ps[:])

            # ---- attention ----
            def emit_norm(c_ps, j, par, q0):
                # ctx^T rows /= row 64 (ones-col sums). Broadcast the sums
                # from psum partition 64 to 0:64 with a K=1 ones matmul.
                # The broadcast lands in the sps rotation (not scr) so the
                # proj/outproj chains keep two conflict-free banks.
                # scalar engine does the 1-row evacuation: ACT is idle at
                # slab boundaries and this keeps the DVE queue clear
                lrow = smp.tile([128, NBW], BF16, tag="lrow", name="lrow")
                nc.scalar.copy(lrow[DH:DH + 1, :], c_ps[DH:DH + 1, :])
                rb_t = psp.tile([128, 2 * NBW], F32, tag="sps", name="rbps",
                                bufs=2)
                rb_ps = rb_t[0:DH, 0:NBW]
                nc.tensor.matmul(rb_ps, ones_t[DH:DH + 1, 0:DH],
                                 lrow[DH:DH + 1, :], start=True, stop=True)
                rb = smp.tile([DH, NBW], F32, tag="rb", name="rb")
                nc.vector.reciprocal_approx_fast(out=rb[:], in_=rb_ps)
                if par == 0:
                    nc.vector.tensor_mul(ctxT[j][0:DH, q0:q0 + NBW],
                                         c_ps[0:DH, :], rb[:])
                else:
                    tmp = smp.tile([DH, NBW], BF16, tag="ctmp", name="ctmp")
                    nc.vector.tensor_mul(tmp[:], c_ps[0:DH, :], rb[:])
                    nc.sync.dma_start(out=ctxT[j][DH:2 * DH, q0:q0 + NBW],
                                      in_=tmp[:])

            def attn(nb):
                q0 = nb * NBW
                n_mb = 4 * nb + 4
                for s in range(4):
                    j, par = s // 2, s % 2
                    c0 = psp.tile([DH + 1, NBW], F32, tag="cps", name="c0",
                                  bufs=2)
                    c1 = psp.tile([DH + 1, NBW], F32, tag="cps", name="c1",
                                  bufs=2)
                    # exp groups: each group = one sps tile + one activation.
                    # The two smallest diagonal blocks (w=256,128) share a
                    # tile: [mb2kv0 0:256|mb3kv0 256:384|mb2kv1 512:768|
                    # mb3kv1 768:896] — each matmul output stays in one bank.
                    groups = [[mb] for mb in range(n_mb - 2)]
                    groups.append([n_mb - 2, n_mb - 1])
                    for grp in groups:
                        place = []
                        if len(grp) == 1:
                            mb = grp[0]
                            off = max(0, mb * 128 - q0)
                            w = NBW - off
                            place.append((mb, off, w, 0, NBW))
                            vw = w
                        else:
                            # kv0 regions stay in bank 0, kv1 in bank 1, so
                            # the concurrently-draining row-tiled pair never
                            # writes the same psum bank.
                            mb2, mb3 = grp
                            place.append((mb2, 256, 256, 0, NBW))
                            place.append((mb3, 384, 128, 256, NBW + 256))
                            vw = 384
                        sp = psp.tile([128, 2 * NBW], F32, tag="sps",
                                      name="sps", bufs=2)
                        for mb, off, w, ca, cb in place:
                            m0 = mb * 128
                            nc.tensor.matmul(
                                sp[:, ca:ca + w],
                                kt[0:DH, m0:m0 + 128],
                                qt[s][0:DH, q0 + off:q0 + NBW],
                                start=True, stop=True)
                            nc.tensor.matmul(
                                sp[:, cb:cb + w],
                                kt[DH:128, m0:m0 + 128],
                                qt[s][DH:128, q0 + off:q0 + NBW],
                                start=True, stop=True)
                        p = ptp.tile([128, 2 * NBW], BF16, tag="pt",
                                     name="pt")
                        if vw == NBW:
                            nc.scalar.activation(p[:, 0:2 * NBW],
                                                 sp[:, 0:2 * NBW],
                                                 EXP, scale=float(scale))
                        else:
                            # strided 2-chunk view skips the stale columns
                            # [vw:NBW] (never written in this tile)
                            sp_v = sp[:].rearrange(
                                "p (c k) -> p c k", c=2)[:, :, 0:vw]
                            p_v = p[:].rearrange(
                                "p (c k) -> p c k", c=2)[:, :, 0:vw]
                            nc.scalar.activation(p_v, sp_v, EXP,
                                                 scale=float(scale))
                        for mb, off, w, ca, cb in place:
                            if mb >= 4 * nb:  # diagonal block
                                nc.vector.tensor_mul(p[:, ca:ca + 128],
                                                     p[:, ca:ca + 128], tri[:])
                                nc.vector.tensor_mul(p[:, cb:cb + 128],
                                                     p[:, cb:cb + 128], tri[:])
                            st, sp_ = (mb == 0), (mb == n_mb - 1)
                            nc.tensor.matmul(c0[:, off:NBW],
                                             vaug[mb][:, 0:DH + 1],
                                             p[:, ca:ca + w],
                                             start=st, stop=sp_)
                            nc.tensor.matmul(c1[:, off:NBW],
                                             vaug[mb][:, DH + 1:2 * (DH + 1)],
                                             p[:, cb:cb + w],
                                             start=st, stop=sp_)
                    emit_norm(c0, j, par, q0)
                    emit_norm(c1, 2 + j, par, q0)

            # ---- out projection ----
            def outproj(nb):
                if nb == NB - 1:
                    # pure tail (attention done): all four freed psum banks
                    # accumulate in parallel, j-outer so each ctxT slice is
                    # loaded once per 4 matmuls; per-chunk DMAs drain early;
                    # copies alternate scalar/vector to halve the drain.
                    for nt in range(4 * nb, 4 * nb + 4):
                        o_sb = outp.tile([128, D], BF16, tag="osb",
                                         name="osb")
                        ops = []
                        for tg in ("cps", "cps", "scr", "prps"):
                            ops.append(psp.tile(
                                [128, NBW], F32, tag=tg, name="ops",
                                bufs=2 if tg == "cps" else 1))
                        for j in range(4):
                            for ob in range(4):
                                nc.tensor.matmul(
                                    ops[ob][:],
                                    ctxT[j][:, nt * 128:(nt + 1) * 128],
                                    wo_sb(j, ob),
                                    start=(j == 0), stop=(j == 3))
                        for ob in range(4):
                            dst = o_sb[:, ob * NBW:(ob + 1) * NBW]
                            if ob % 2 == 0:
                                nc.scalar.copy(dst, ops[ob][:])
                            else:
                                nc.vector.tensor_copy(dst, ops[ob][:])
                            nc.sync.dma_start(
                                out=OUT[nt * 128:(nt + 1) * 128,
                                        ob * NBW:(ob + 1) * NBW],
                                in_=o_sb[:, ob * NBW:(ob + 1) * NBW])
                    return
                # filler for the attn(3) window (no proj runs there);
                # chains alternate scr/prps banks.
                u = 0
                for nt in range(4 * nb, 4 * nb + 4):
                    o_sb = outp.tile([128, D], BF16, tag="osb", name="osb")
                    for ob in range(4):
                        tg = "scr" if u % 2 == 0 else "prps"
                        u += 1
                        o_ps = psp.tile([128, NBW], F32, tag=tg, name="ops",
                                        bufs=1)
                        for j in range(4):
                            nc.tensor.matmul(
                                o_ps[:],
                                ctxT[j][:, nt * 128:(nt + 1) * 128],
                                wo_sb(j, ob),
                                start=(j == 0), stop=(j == 3))
                        nc.vector.tensor_copy(
                            o_sb[:, ob * NBW:(ob + 1) * NBW], o_ps[:])
                    nc.sync.dma_start(out=OUT[nt * 128:(nt + 1) * 128, :],
                                      in_=o_sb[:])

            # ---- program: per-q-block pipeline ----
            # proj(nb+1) sits after attn(nb) in program order (= lower
            # scheduler priority) so it fills PE idle while ACT churns.
            # outproj(0..2) deferred after attn(3): that window is ACT-bound
            # and otherwise short of PE filler.
            proj_k(0)
            for s in range(4):
                proj_q(0, s)
            proj_v(0)
            for nb in range(NB):
                attn(nb)
                if nb + 1 < NB:
                    proj_k(nb + 1)
                    for s in range(4):
                        proj_q(nb + 1, s)
                    proj_v(nb + 1)
            outproj(0)
            outproj(1)
            outproj(2)
            outproj(NB - 1)

    nc.compile()
    return nc


def kernel(x, Wq, Wk, Wv, Wo, bo):
    x = np.asarray(x, dtype=np.float32)
    Wq = np.asarray(Wq, dtype=np.float32)
    Wk = np.asarray(Wk, dtype=np.float32)
    Wv = np.asarray(Wv, dtype=np.float32)
    Wo = np.asarray(Wo, dtype=np.float32)
    bo = np.asarray(bo, dtype=np.float32)
    bf = ml_dtypes.bfloat16

    if "nc" not in _CACHED:
        _CACHED["nc"] = _build()
    nc = _CACHED["nc"]

    in_maps = []
    for c in range(8):
        b, t = c // 4, c % 4
        xT = np.ascontiguousarray(x[b].T).astype(bf)
        # q slab s holds [kv-head 2t head g=s | kv-head 2t+1 head g=s]
        qcols = []
        for s in range(4):
            for kvl in range(KVPC):
                h = (2 * t + kvl) * G + s
                qcols.append(Wq[:, h * DH:(h + 1) * DH])
        wq_c = np.ascontiguousarray(np.concatenate(qcols, axis=1)).astype(bf)
        wk_c = np.ascontiguousarray(Wk[:, t * 128:(t + 1) * 128]).astype(bf)
        wv_c = np.ascontiguousarray(Wv[:, t * 128:(t + 1) * 128]).astype(bf)
        wo_c = np.ascontiguousarray(Wo[t * DQ:(t + 1) * DQ, :]).astype(bf)
        in_maps.append({"xT": xT, "Wq": wq_c, "Wk": wk_c, "Wv": wv_c,
                        "Wo": wo_c})

    trace = bool(int(os.environ.get("GQA_TRACE", "0")))
    kwargs = {}
    if trace:
        import tempfile
        td = os.environ.get("GQA_TRACE_DIR") or tempfile.mkdtemp(prefix="gqa_")
        kwargs = dict(trace=True, tmpdir=td)
    res = run_bass_kernel_spmd(nc, in_maps, list(range(8)), **kwargs)
    _CACHED["last_result"] = res

    out = np.empty((B, N, D), dtype=np.float32)
    for b in range(B):
        acc = res.results[4 * b]["out"].astype(np.float32)
        for t in range(1, 4):
            acc = acc + res.results[4 * b + t]["out"].astype(np.float32)
        out[b] = acc + bo[None, :]
    return out


# revision 27
# speedup vs baseline: 1.0511x; 1.0203x over previous
"""GQA forward (B=2,N=2048,D=2048,H=32,KV=8,DH=64, causal) on 8 trn2 cores.

Sharding: 2-way data parallel over batch x 4-way tensor parallel over heads
(each core: 8 q-heads = 2 kv-heads, keeping group structure). Row-parallel
out-proj; the all-reduce over the 4 TP shards (+ bias) happens on host at
gather time.

v5 = v2 structure (HW-proven numerics paths only) + trace-driven fixes:
  - batched input DMAs: one dma_start per tensor phase (packets spread
    across all 16 DMA engines; x block 0 split across the sync+scalar
    HWDGE queues), single-tile weight/x buffers
  - dependency-free garbage warmup matmuls from t~0 sized to cover the
    input-DMA window
  - exp reads a strided 2-chunk view on partial key blocks (skips the
    never-written columns; slightly fewer ACT cycles, race-check clean)
  - outproj(0,1) deferred after attn(3) program-order: that window is
    ACT-bound (57us exp vs 37us attention matmul) and has no proj filler
  - final outproj psum evacuation alternates scalar/vector engines

Device kernel (per core):
  proj:  streaming projections from xT (host-pretransposed, bf16):
         Q^T (4 pair-slabs), K^T, V^T -> V (PE transpose) into vaug with
         ones columns (softmax row-sums for free in the ctx matmul)
  attn:  per pair (g, kv0/kv1), per 128-key block: two row-tiled scores
         matmuls S^T = K^T.T @ Q^T into one 2-bank PSUM tile, one exp on
         ACT (scale=1/sqrt(dh)), triangle-mask multiply on diagonal
         blocks, ctx^T accumulated per head in PSUM with row 64 = softmax
         denominator; normalize on the PSUM->SBUF copy.
  out:   out = ctx @ Wo_shard accumulated over 4 contraction chunks,
         written bf16; host sums the 4 TP partials + bias in fp32.
"""
import os
import sys

import numpy as np

if "/opt/trn_rl_repo" not in sys.path:
    sys.path.insert(0, "/opt/trn_rl_repo")

import ml_dtypes

import concourse.bacc as bacc
import concourse.tile as tile
from concourse import mybir
from concourse.bass_utils import run_bass_kernel_spmd
from concourse.masks import make_identity

F32 = mybir.dt.float32
BF16 = mybir.dt.bfloat16
EXP = mybir.ActivationFunctionType.Exp

B, N, D = 2, 2048, 2048
H, KV, DH = 32, 8, 64
G = H // KV                      # 4 q-heads per kv head
HPC, KVPC = 8, 2                 # heads / kv-heads per core
DQ = HPC * DH                    # 512 per-core q projection width
NBW = 512                        # q-block width for attention
NB = N // NBW                    # 4 q-blocks
DC = D // 128                    # 16 contraction chunks
NT = N // 128                    # 16 row tiles
NWARM = 64                       # garbage warmup matmuls (HAM + DMA cover)

_CACHED = {}


def _build():
    nc = bacc.Bacc("TRN2", target_bir_lowering=False, debug=False, num_devices=8)

    xT = nc.dram_tensor("xT", [D, N], BF16, kind="ExternalInput")
    Wq = nc.dram_tensor("Wq", [D, DQ], BF16, kind="ExternalInput")
    Wk = nc.dram_tensor("Wk", [D, KVPC * DH], BF16, kind="ExternalInput")
    Wv = nc.dram_tensor("Wv", [D, KVPC * DH], BF16, kind="ExternalInput")
    Wo = nc.dram_tensor("Wo", [DQ, D], BF16, kind="ExternalInput")
    OUT = nc.dram_tensor("out", [N, D], BF16, kind="ExternalOutput")

    scale = 1.0 / np.sqrt(DH)

    with tile.TileContext(nc) as tc:
        with (
            tc.tile_pool(name="persist", bufs=1) as pp,
            tc.tile_pool(name="vt", bufs=2) as vtp,
            tc.tile_pool(name="pt", bufs=6) as ptp,
            tc.tile_pool(name="outs", bufs=2) as outp,
            tc.tile_pool(name="small", bufs=3) as smp,
            tc.tile_pool(name="ps", bufs=1, space="PSUM") as psp,
        ):
            # ---- PE warmup: dependency-free garbage matmuls keep the HAM
            # clock warming while the batched input DMAs stream in ----
            junk = pp.tile([128, NBW], BF16, tag="junk")
            nc.vector.memset(junk[:], 0.0)
            wu_ps = psp.tile([128, NBW], F32, tag="sps", name="wu", bufs=2)
            for _ in range(NWARM):
                nc.tensor.matmul(wu_ps[:], junk[:, 0:128], junk[:],
                                 start=True, stop=True)

            # ---- batched input DMAs: one instruction per tensor phase ----
            xs_all = pp.tile([128, DC * N], BF16, tag="xsall")
            wk_all = pp.tile([128, DC * KVPC * DH], BF16, tag="wkall")
            wv_all = pp.tile([128, DC * KVPC * DH], BF16, tag="wvall")
            wq_all = pp.tile([128, DC * DQ], BF16, tag="wqall")
            wo_all = pp.tile([128, 4 * D], BF16, tag="woall")

            xs_v = xs_all[:].rearrange("p (dc n) -> p dc n", dc=DC)
            xT_v = xT[:, :].rearrange("(dc p) n -> p dc n", p=128)
            # critical path first: wk + x block 0 + wq gate proj(0).
            # Asymmetric x0 split: scalar's small quarter clears fast so
            # wq (behind it in the ring) lands right as proj_k finishes;
            # x block 1 is its own transfer so proj(1) isn't gated on the
            # 4MB bulk. ~375GB/s aggregate across the 16 DMA engines.
            nc.sync.dma_start(
                out=wk_all[:].rearrange("p (dc m) -> p dc m", dc=DC),
                in_=Wk[:, :].rearrange("(dc p) m -> p dc m", p=128))
            # x0 chunked so proj_k's accumulation chain is paced by DMA
            # arrival (dc 0-3 lands ~14us) instead of one big gate
            for d0, d1 in ((0, 4), (4, 8), (8, 11)):
                nc.sync.dma_start(out=xs_v[:, d0:d1, 0:NBW],
                                  in_=xT_v[:, d0:d1, 0:NBW])
            nc.scalar.dma_start(out=xs_v[:, 11:DC, 0:NBW],
                                in_=xT_v[:, 11:DC, 0:NBW])
            nc.scalar.dma_start(
                out=wq_all[:].rearrange("p (dc m) -> p dc m", dc=DC),
                in_=Wq[:, :].rearrange("(dc p) m -> p dc m", p=128))
            nc.sync.dma_start(out=xs_v[:, :, NBW:2 * NBW],
                              in_=xT_v[:, :, NBW:2 * NBW])
            nc.sync.dma_start(
                out=wv_all[:].rearrange("p (dc m) -> p dc m", dc=DC),
                in_=Wv[:, :].rearrange("(dc p) m -> p dc m", p=128))
            nc.sync.dma_start(out=xs_v[:, :, 2 * NBW:N],
                              in_=xT_v[:, :, 2 * NBW:N])
            nc.scalar.dma_start(
                out=wo_all[:].rearrange("p (j d) -> p j d", j=4),
                in_=Wo[:, :].rearrange("(j p) d -> p j d", p=128))

            def xs(dc, nb):
                return xs_all[:, dc * N + nb * NBW:dc * N + (nb + 1) * NBW]

            def wk_sb(dc):
                return wk_all[:, dc * 128:(dc + 1) * 128]

            def wv_sb(dc):
                return wv_all[:, dc * 128:(dc + 1) * 128]

            def wq_sb(dc, s):
                return wq_all[:, dc * DQ + s * 128:dc * DQ + (s + 1) * 128]

            def wo_sb(j, ob):
                return wo_all[:, j * D + ob * NBW:j * D + (ob + 1) * NBW]

            # ---- persistent sbuf state ----
            ident_f = pp.tile([128, 128], F32, tag="identf")
            make_identity(nc, ident_f[:])
            ident = pp.tile([128, 128], BF16, tag="ident")
            nc.vector.tensor_copy(ident[:], ident_f[:])

            # triangle mask: tri[r, j] = 1 if j >= r else 0
            tri_f = pp.tile([128, 128], F32, tag="trif")
            nc.gpsimd.memset(tri_f[:], 1.0)
            nc.gpsimd.affine_select(
                out=tri_f[:], in_=tri_f[:],
                compare_op=mybir.AluOpType.is_ge,
                fill=0.0, base=0,
                pattern=[[1, 128]],
                channel_multiplier=-1,
            )
            tri = pp.tile([128, 128], BF16, tag="tri")
            nc.vector.tensor_copy(tri[:], tri_f[:])

            ones_t = pp.tile([128, DH], BF16, tag="ones")
            nc.vector.memset(ones_t[:], 1.0)

            # q slabs: slab s = [kv0 g=s (rows 0:64) | kv1 g=s (rows 64:128)]
            qt = [pp.tile([128, N], BF16, tag=f"qt{s}", name=f"qt{s}")
                  for s in range(4)]
            kt = pp.tile([128, N], BF16, tag="kt")
            # vaug[m]: [0:64]=v_kv0, 64=ones, [65:129]=v_kv1, 129=ones
            vaug = [pp.tile([128, 2 * (DH + 1)], BF16, tag=f"va{m}",
                            name=f"va{m}") for m in range(NT)]
            for m in range(NT):
                nc.vector.memset(vaug[m][:], 1.0)
            ctxT = [pp.tile([128, N], BF16, tag=f"ct{j}", name=f"ct{j}")
                    for j in range(4)]

            # ---- projection helpers (chains alternate scr/prps so chain
            # i+1 never waits on chain i's evacuation cast) ----
            def proj_k(nb):
                ncol = slice(nb * NBW, (nb + 1) * NBW)
                k_ps = psp.tile([128, NBW], F32, tag="scr", name="kps")
                for dc in range(DC):
                    nc.tensor.matmul(k_ps[:], wk_sb(dc), xs(dc, nb),
                                     start=(dc == 0), stop=(dc == DC - 1))
                nc.vector.tensor_copy(kt[:, ncol], k_ps[:])

            def proj_v(nb):
                v_ps = psp.tile([128, NBW], F32, tag="prps", name="vps")
                for dc in range(DC):
                    nc.tensor.matmul(v_ps[:], wv_sb(dc), xs(dc, nb),
                                     start=(dc == 0), stop=(dc == DC - 1))
                vts = vtp.tile([128, NBW], BF16, tag="vts")
                nc.vector.tensor_copy(vts[:], v_ps[:])
                for i in range(NBW // 128):
                    mt = nb * (NBW // 128) + i
                    tp = psp.tile([128, 128], BF16, tag="scr", name="tps")
                    nc.tensor.transpose(tp[:], vts[:, i * 128:(i + 1) * 128],
                                        ident[:])
                    nc.vector.tensor_copy(vaug[mt][:, 0:DH], tp[:, 0:DH])
                    nc.vector.tensor_copy(vaug[mt][:, DH + 1:2 * DH + 1],
                                          tp[:, DH:2 * DH])

            def proj_q(nb, s):
                tg = "prps" if s % 2 == 0 else "scr"
                ncol = slice(nb * NBW, (nb + 1) * NBW)
                q_ps = psp.tile([128, NBW], F32, tag=tg, name="qps")
                for dc in range(DC):
                    nc.tensor.matmul(q_ps[:], wq_sb(dc, s), xs(dc, nb),
                                     start=(dc == 0), stop=(dc == DC - 1))
                nc.vector.tensor_copy(qt[s][:, ncol], q_ps[:])

            # ---- attention ----
            def emit_norm(c_ps, j, par, q0):
                # ctx^T rows /= row 64 (ones-col sums). Broadcast the sums
                # from psum partition 64 to 0:64 with a K=1 ones matmul.
                # The broadcast lands in the sps rotation (not scr) so the
                # proj/outproj chains keep two conflict-free banks.
                # scalar engine does the 1-row evacuation: ACT is idle at
                # slab boundaries and this keeps the DVE queue clear
                lrow = smp.tile([128, NBW], BF16, tag="lrow", name="lrow")
                nc.scalar.copy(lrow[DH:DH + 1, :], c_ps[DH:DH + 1, :])
                rb_t = psp.tile([128, 2 * NBW], F32, tag="sps", name="rbps",
                                bufs=2)
                rb_ps = rb_t[0:DH, 0:NBW]
                nc.tensor.matmul(rb_ps, ones_t[DH:DH + 1, 0:DH],
                                 lrow[DH:DH + 1, :], start=True, stop=True)
                rb = smp.tile([DH, NBW], F32, tag="rb", name="rb")
                nc.vector.reciprocal_approx_fast(out=rb[:], in_=rb_ps)
                if par == 0:
                    nc.vector.tensor_mul(ctxT[j][0:DH, q0:q0 + NBW],
                                         c_ps[0:DH, :], rb[:])
                else:
                    tmp = smp.tile([DH, NBW], BF16, tag="ctmp", name="ctmp")
                    nc.vector.tensor_mul(tmp[:], c_ps[0:DH, :], rb[:])
                    nc.sync.dma_start(out=ctxT[j][DH:2 * DH, q0:q0 + NBW],
                                      in_=tmp[:])

            def attn(nb):
                q0 = nb * NBW
                n_mb = 4 * nb + 4
                for s in range(4):
                    j, par = s // 2, s % 2
                    c0 = psp.tile([DH + 1, NBW], F32, tag="cps", name="c0",
                                  bufs=2)
                    c1 = psp.tile([DH + 1, NBW], F32, tag="cps", name="c1",
                                  bufs=2)
                    # exp groups: each group = one sps tile + one activation.
                    # The two smallest diagonal blocks (w=256,128) share a
                    # tile: [mb2kv0 0:256|mb3kv0 256:384|mb2kv1 512:768|
                    # mb3kv1 768:896] — each matmul output stays in one bank.
                    groups = [[mb] for mb in range(n_mb - 2)]
                    groups.append([n_mb - 2, n_mb - 1])
                    for grp in groups:
                        place = []
                        if len(grp) == 1:
                            mb = grp[0]
                            off = max(0, mb * 128 - q0)
                            w = NBW - off
                            place.append((mb, off, w, 0, NBW))
                            vw = w
                        else:
                            # kv0 regions stay in bank 0, kv1 in bank 1, so
                            # the concurrently-draining row-tiled pair never
                            # writes the same psum bank.
                            mb2, mb3 = grp
                            place.append((mb2, 256, 256, 0, NBW))
                            place.append((mb3, 384, 128, 256, NBW + 256))
                            vw = 384
                        sp = psp.tile([128, 2 * NBW], F32, tag="sps",
                                      name="sps", bufs=2)
                        for mb, off, w, ca, cb in place:
                            m0 = mb * 128
                            nc.tensor.matmul(
                                sp[:, ca:ca + w],
                                kt[0:DH, m0:m0 + 128],
                                qt[s][0:DH, q0 + off:q0 + NBW],
                                start=True, stop=True)
                            nc.tensor.matmul(
                                sp[:, cb:cb + w],
                                kt[DH:128, m0:m0 + 128],
                                qt[s][DH:128, q0 + off:q0 + NBW],
                                start=True, stop=True)
                        p = ptp.tile([128, 2 * NBW], BF16, tag="pt",
                                     name="pt")
                        if vw == NBW:
                            nc.scalar.activation(p[:, 0:2 * NBW],
                                                 sp[:, 0:2 * NBW],
                                                 EXP, scale=float(scale))
                        else:
                            # strided 2-chunk view skips the stale columns
                            # [vw:NBW] (never written in this tile)
                            sp_v = sp[:].rearrange(
                                "p (c k) -> p c k", c=2)[:, :, 0:vw]
                            p_v = p[:].rearrange(
                                "p (c k) -> p c k", c=2)[:, :, 0:vw]
                            nc.scalar.activation(p_v, sp_v, EXP,
                                                 scale=float(scale))
                        for mb, off, w, ca, cb in place:
                            if mb >= 4 * nb:  # diagonal block
                                nc.vector.tensor_mul(p[:, ca:ca + 128],
                                                     p[:, ca:ca + 128], tri[:])
                                nc.vector.tensor_mul(p[:, cb:cb + 128],
                                                     p[:, cb:cb + 128], tri[:])
                            st, sp_ = (mb == 0), (mb == n_mb - 1)
                            nc.tensor.matmul(c0[:, off:NBW],
                                             vaug[mb][:, 0:DH + 1],
                                             p[:, ca:ca + w],
                                             start=st, stop=sp_)
                            nc.tensor.matmul(c1[:, off:NBW],
                                             vaug[mb][:, DH + 1:2 * (DH + 1)],
                                             p[:, cb:cb + w],
                                             start=st, stop=sp_)
                    emit_norm(c0, j, par, q0)
                    emit_norm(c1, 2 + j, par, q0)

            # ---- out projection ----
            def outproj(nb):
                if nb == NB - 1:
                    # pure tail (attention done): all four freed psum banks
                    # accumulate in parallel, j-outer so each ctxT slice is
                    # loaded once per 4 matmuls; per-chunk DMAs drain early;
                    # copies alternate scalar/vector to halve the drain.
                    for nt in range(4 * nb, 4 * nb + 4):
                        o_sb = outp.tile([128, D], BF16, tag="osb",
                                         name="osb")
                        ops = []
                        for tg in ("cps", "cps", "scr", "prps"):
                            ops.append(psp.tile(
                                [128, NBW], F32, tag=tg, name="ops",
                                bufs=2 if tg == "cps" else 1))
                        for j in range(4):
                            for ob in range(4):
                                nc.tensor.matmul(
                                    ops[ob][:],
                                    ctxT[j][:, nt * 128:(nt + 1) * 128],
                                    wo_sb(j, ob),
                                    start=(j == 0), stop=(j == 3))
                        for ob in range(4):
                            dst = o_sb[:, ob * NBW:(ob + 1) * NBW]
                            if ob % 2 == 0:
                                nc.scalar.copy(dst, ops[ob][:])
                            else:
                                nc.vector.tensor_copy(dst, ops[ob][:])
                            nc.sync.dma_start(
                                out=OUT[nt * 128:(nt + 1) * 128,
                                        ob * NBW:(ob + 1) * NBW],
                                in_=o_sb[:, ob * NBW:(ob + 1) * NBW])
                    return
                # filler for the attn(3) window (no proj runs there);
                # chains alternate scr/prps banks.
                u = 0
                for nt in range(4 * nb, 4 * nb + 4):
                    o_sb = outp.tile([128, D], BF16, tag="osb", name="osb")
                    for ob in range(4):
                        tg = "scr" if u % 2 == 0 else "prps"
                        u += 1
                        o_ps = psp.tile([128, NBW], F32, tag=tg, name="ops",
                                        bufs=1)
                        for j in range(4):
                            nc.tensor.matmul(
                                o_ps[:],
                                ctxT[j][:, nt * 128:(nt + 1) * 128],
                                wo_sb(j, ob),
                                start=(j == 0), stop=(j == 3))
                        nc.vector.tensor_copy(
                            o_sb[:, ob * NBW:(ob + 1) * NBW], o_ps[:])
                    nc.sync.dma_start(out=OUT[nt * 128:(nt + 1) * 128, :],
                                      in_=o_sb[:])

            # ---- program: per-q-block pipeline ----
            # proj(nb+1) sits after attn(nb) in program order (= lower
            # scheduler priority) so it fills PE idle while ACT churns.
            # outproj(0..2) deferred after attn(3): that window is ACT-bound
            # and otherwise short of PE filler.
            proj_k(0)
            for s in range(4):
                proj_q(0, s)
            proj_v(0)
            for nb in range(NB):
                attn(nb)
                if nb + 1 < NB:
                    proj_k(nb + 1)
                    for s in range(4):
                        proj_q(nb + 1, s)
                    proj_v(nb + 1)
            outproj(0)
            outproj(1)
            outproj(2)
            outproj(NB - 1)

    nc.compile()
    return nc


def kernel(x, Wq, Wk, Wv, Wo, bo):
    x = np.asarray(x, dtype=np.float32)
    Wq = np.asarray(Wq, dtype=np.float32)
    Wk = np.asarray(Wk, dtype=np.float32)
    Wv = np.asarray(Wv, dtype=np.float32)
    Wo = np.asarray(Wo, dtype=np.float32)
    bo = np.asarray(bo, dtype=np.float32)
    bf = ml_dtypes.bfloat16

    if "nc" not in _CACHED:
        _CACHED["nc"] = _build()
    nc = _CACHED["nc"]

    in_maps = []
    for c in range(8):
        b, t = c // 4, c % 4
        xT = np.ascontiguousarray(x[b].T).astype(bf)
        # q slab s holds [kv-head 2t head g=s | kv-head 2t+1 head g=s]
        qcols = []
        for s in range(4):
            for kvl in range(KVPC):
                h = (2 * t + kvl) * G + s
                qcols.append(Wq[:, h * DH:(h + 1) * DH])
        wq_c = np.ascontiguousarray(np.concatenate(qcols, axis=1)).astype(bf)
        wk_c = np.ascontiguousarray(Wk[:, t * 128:(t + 1) * 128]).astype(bf)
        wv_c = np.ascontiguousarray(Wv[:, t * 128:(t + 1) * 128]).astype(bf)
        wo_c = np.ascontiguousarray(Wo[t * DQ:(t + 1) * DQ, :]).astype(bf)
        in_maps.append({"xT": xT, "Wq": wq_c, "Wk": wk_c, "Wv": wv_c,
                        "Wo": wo_c})

    trace = bool(int(os.environ.get("GQA_TRACE", "0")))
    kwargs = {}
    if trace:
        import tempfile
        td = os.environ.get("GQA_TRACE_DIR") or tempfile.mkdtemp(prefix="gqa_")
        kwargs = dict(trace=True, tmpdir=td)
    res = run_bass_kernel_spmd(nc, in_maps, list(range(8)), **kwargs)
    _CACHED["last_result"] = res

    out = np.empty((B, N, D), dtype=np.float32)
    for b in range(B):
        acc = res.results[4 * b]["out"].astype(np.float32)
        for t in range(1, 4):
            acc = acc + res.results[4 * b + t]["out"].astype(np.float32)
        out[b] = acc + bo[None, :]
    return out
